# revision 1
# baseline (speedup 1.0000x reference)
"""Multi-head low-rank-score (LSR) causal attention on 8 trn2 NeuronCores.

Sharding: core = 4*b + g owns batch b and heads [4g, 4g+4).
Each core: q/k/v projections for its 256 head-dims, low-rank score
projections, causal softmax attention, and a partial o_proj
(its 256 ctx dims x full 1024 out dims). Host sums the 4 partials
per batch and adds biases.

All matmuls run in float32r (full-rate fp32, ~13-bit mantissa).
Softmax: two score passes -
  pass 1 (q-layout) computes the per-query causal row max on DVE;
  pass 2 (k-layout) computes S^T with the max subtraction and the
  block-level causal mask folded into the matmul via augmented
  contraction rows, then exp() on ScalarE writes P^T straight to SBUF.
AV multiplies V (augmented with a ones column -> softmax denominators
come out of the same matmul) by P^T, and the division is applied via a
reciprocal + rank-2 broadcast matmul + one DVE multiply.
"""

import numpy as np
import ml_dtypes

B = 2
T = 2048
D = 1024
H = 16
DH = 64
R = 32
HPC = 4  # heads per core
OC = HPC * DH  # 256 out-cols per core
NCORES = 8
SCALE = 1.0 / float(np.sqrt(np.float32(R)))
NEG = -30000.0
NT = T // 128  # 16 key/query tiles
NCH = T // 512  # 4 query chunks

_cache = {}


def _build():
    import concourse.bacc as bacc
    import concourse.mybir as mybir
    from concourse.tile import TileContext

    F32 = mybir.dt.float32
    F32R = mybir.dt.float32r
    BF16 = mybir.dt.bfloat16
    EXP = mybir.ActivationFunctionType.Exp
    COPY = mybir.ActivationFunctionType.Copy
    IDENT = mybir.ActivationFunctionType.Identity
    MAX = mybir.AluOpType.max
    AXX = mybir.AxisListType.X

    nc = bacc.Bacc("TRN2", target_bir_lowering=False, debug=False,
                   num_devices=NCORES)

    xT = nc.declare_dram_parameter("xT", [D, T], F32R, isOutput=False)
    wq = nc.declare_dram_parameter("wq", [D, OC], F32R, isOutput=False)
    wk = nc.declare_dram_parameter("wk", [D, OC], F32R, isOutput=False)
    wv = nc.declare_dram_parameter("wv", [D, OC], F32R, isOutput=False)
    wo = nc.declare_dram_parameter("wo", [OC, D], BF16, isOutput=False)
    wql = nc.declare_dram_parameter("wql", [DH, HPC * R], F32R, isOutput=False)
    wkl = nc.declare_dram_parameter("wkl", [DH, HPC * R], F32R, isOutput=False)
    bq = nc.declare_dram_parameter("bq", [OC, 1], F32, isOutput=False)
    bk = nc.declare_dram_parameter("bk", [OC, 1], F32, isOutput=False)
    # [16, T] row j': NEG where t < 128*j' else 0
    indq = nc.declare_dram_parameter("indq", [NT, T], F32R, isOutput=False)
    # [17, T]: row 0 = ones; rows 1+j': 1.0 on k-tile j' cols else 0
    okq = nc.declare_dram_parameter("okq", [NT + 1, T], F32R, isOutput=False)
    triq = nc.declare_dram_parameter("triq", [128, 128], F32, isOutput=False)
    trik = nc.declare_dram_parameter("trik", [128, 128], F32, isOutput=False)
    sel2 = nc.declare_dram_parameter("sel2", [2, 128], F32R, isOutput=False)
    ibf = nc.declare_dram_parameter("ibf", [128, 128], BF16, isOutput=False)
    yT = nc.declare_dram_parameter("yT", [D, T], F32, isOutput=True)

    with TileContext(nc) as tc:
        with (
            nc.allow_low_precision(reason="f32r reciprocal / bf16 row-max"),
            tc.tile_pool(name="persist", bufs=1) as pp,
        ):
            # ---- persistent SBUF tiles
            wq_t = [pp.tile([128, OC], F32R, tag=f"wq{i}", name=f"wq{i}") for i in range(8)]
            wk_t = [pp.tile([128, OC], F32R, tag=f"wk{i}", name=f"wk{i}") for i in range(8)]
            wv_t = [pp.tile([128, OC], F32R, tag=f"wv{i}", name=f"wv{i}") for i in range(8)]
            # lsr weights duplicated at partition bases 0 and 64 (row packing)
            wql_t = pp.tile([128, HPC * R], F32R, tag="wql")
            wkl_t = pp.tile([128, HPC * R], F32R, tag="wkl")
            bq_t = [pp.tile([128, 1], F32, tag=f"bq{i}", name=f"bq{i}") for i in range(2)]
            bk_t = [pp.tile([128, 1], F32, tag=f"bk{i}", name=f"bk{i}") for i in range(2)]
            triq_t = pp.tile([128, 128], F32, tag="triq")
            trik_t = pp.tile([128, 128], F32, tag="trik")
            sel2_t = pp.tile([2, 128], F32R, tag="sel2")
            ibf_t = pp.tile([128, 128], BF16, tag="ibf")
            # augmented lr tiles, one per head pair p (heads 2p, 2p+1)
            # rows [64l, 64l+32): scale*q_lrT / k_lrT of head 2p+l
            # row 64l+32: -m (q side) / ones (k side)
            # rows [64l+33, 64l+49): indq (q side) / selk (k side)
            qaug = [pp.tile([128, T], F32R, tag=f"qaug{p}", name=f"qaug{p}") for p in range(2)]
            kaug = [pp.tile([128, T], F32R, tag=f"kaug{p}", name=f"kaug{p}") for p in range(2)]
            # V augmented with ones column, per head x key tile
            vaug = [[pp.tile([128, DH + 1], BF16, tag=f"va{h}_{j}", name=f"va{h}_{j}")
                     for j in range(NT)] for h in range(HPC)]
            # scaled ctx^T ready for o_proj: [pair][chunk]
            ctxr = [[pp.tile([128, 512], BF16, tag=f"cx{p}_{c}", name=f"cx{p}_{c}")
                     for c in range(NCH)] for p in range(2)]
            wo_t = [pp.tile([128, D], BF16, tag=f"wo{p}", name=f"wo{p}") for p in range(2)]

            for i in range(8):
                nc.sync.dma_start(out=wq_t[i][:], in_=wq[128 * i:128 * i + 128, :])
                nc.sync.dma_start(out=wk_t[i][:], in_=wk[128 * i:128 * i + 128, :])
                nc.sync.dma_start(out=wv_t[i][:], in_=wv[128 * i:128 * i + 128, :])
            for l in range(2):
                nc.sync.dma_start(out=wql_t[64 * l:64 * l + DH, :], in_=wql[:])
                nc.sync.dma_start(out=wkl_t[64 * l:64 * l + DH, :], in_=wkl[:])
            for i in range(2):
                nc.sync.dma_start(out=bq_t[i][:], in_=bq[128 * i:128 * i + 128, :])
                nc.sync.dma_start(out=bk_t[i][:], in_=bk[128 * i:128 * i + 128, :])
            nc.sync.dma_start(out=triq_t[:], in_=triq[:])
            nc.sync.dma_start(out=trik_t[:], in_=trik[:])
            nc.sync.dma_start(out=sel2_t[:], in_=sel2[:])
            nc.sync.dma_start(out=ibf_t[:], in_=ibf[:])
            for p in range(2):
                nc.sync.dma_start(out=wo_t[p][:], in_=wo[128 * p:128 * p + 128, :])
                for l in range(2):
                    nc.sync.dma_start(out=qaug[p][64 * l + 33:64 * l + 49, :],
                                      in_=indq[:])
                    nc.sync.dma_start(out=kaug[p][64 * l + 32:64 * l + 49, :],
                                      in_=okq[:])

            # ---- phase 1: projections (uses xT; QT/KT transient)
            with (
                tc.tile_pool(name="px", bufs=1) as px,
                tc.tile_pool(name="pqk", bufs=2) as pqk,
                tc.tile_pool(name="ps1", bufs=2, space="PSUM") as ps1,
                tc.tile_pool(name="psl", bufs=2, space="PSUM") as psl,
            ):
                xt_t = [px.tile([128, T], F32R, tag=f"x{i}", name=f"x{i}") for i in range(8)]
                for i in range(8):
                    nc.sync.dma_start(out=xt_t[i][:],
                                      in_=xT[128 * i:128 * i + 128, :])

                # V: [t-tile, 256] accumulating 8 k-tiles
                for tt in range(NT):
                    vps = ps1.tile([128, OC], F32, tag="vps")
                    for kk in range(8):
                        nc.tensor.matmul(
                            vps[:], xt_t[kk][:, 128 * tt:128 * tt + 128],
                            wv_t[kk][:], start=(kk == 0), stop=(kk == 7))
                    for h in range(HPC):
                        nc.scalar.copy(vaug[h][tt][:, 0:DH],
                                       vps[:, DH * h:DH * h + DH])
                        nc.vector.memset(vaug[h][tt][:, DH:DH + 1], 1.0)

                # QT/KT oc-tiles -> lsr -> aug tiles; QT/KT slots recycled
                for side in range(2):  # 0 = q, 1 = k
                    w_t = wq_t if side == 0 else wk_t
                    b_t = bq_t if side == 0 else bk_t
                    lsr_w = wql_t if side == 0 else wkl_t
                    aug = qaug if side == 0 else kaug
                    evac_scale = SCALE if side == 0 else 1.0
                    for ot in range(2):  # oc tile = head pair p = ot
                        qk_sb = pqk.tile([128, T], F32R, tag="qkt")
                        for ch in range(NCH):
                            pps = ps1.tile([128, 512], F32, tag="pps")
                            for kk in range(8):
                                nc.tensor.matmul(
                                    pps[:],
                                    w_t[kk][:, 128 * ot:128 * ot + 128],
                                    xt_t[kk][:, 512 * ch:512 * ch + 512],
                                    start=(kk == 0), stop=(kk == 7))
                            nc.scalar.activation(
                                qk_sb[:, 512 * ch:512 * ch + 512], pps[:],
                                IDENT, bias=b_t[ot][:], scale=1.0)
                        # lsr for the two heads in this oc tile (row-packed
                        # at partition bases 0 / 64)
                        for ch in range(NCH):
                            for l in range(2):
                                h = 2 * ot + l
                                lps = psl.tile([R, 512], F32, tag=f"lps{l}", name=f"lps{l}")
                                nc.tensor.matmul(
                                    lps[:],
                                    lsr_w[64 * l:64 * l + DH,
                                          R * h:R * h + R],
                                    qk_sb[64 * l:64 * l + DH,
                                          512 * ch:512 * ch + 512],
                                    start=True, stop=True,
                                    tile_position=(64 * l, 0))
                                nc.scalar.activation(
                                    aug[ot][64 * l:64 * l + R,
                                            512 * ch:512 * ch + 512],
                                    lps[:], COPY, scale=evac_scale)

            # ---- phases 2-4 merged: stats / S^T+AV / o_proj interleaved
            # per 512-query chunk so the PE never idles long enough to
            # re-throttle. PSUM: sT0(2) + sT1(1) + st0/st1(2) + av0/av1(2)
            # + nmt(1) = 8 banks; scl shares st0, yps shares st1.
            with (
                tc.tile_pool(name="psw", bufs=1, space="PSUM") as psw,
                tc.tile_pool(name="psT", bufs=2, space="PSUM") as psT,
                tc.tile_pool(name="psav", bufs=1, space="PSUM") as psav,
                tc.tile_pool(name="pmx", bufs=2) as pmx,
                tc.tile_pool(name="pst", bufs=6) as pst,
                tc.tile_pool(name="pcx", bufs=2) as pcx,
            ):
                def emit_stats(c):
                    for ii in range(4):
                        i = 4 * c + ii
                        nchunks = i // 4 + 1
                        mx = pmx.tile([128, HPC * 4], F32, tag="mx",
                                      name="mx")
                        negm = pmx.tile([128, 32], BF16, tag="negm",
                                        name="negm")
                        for p in range(2):
                            for l in range(2):
                                h = 2 * p + l
                                for cc in range(nchunks):
                                    ncols = min(512, 128 * (i + 1) - 512 * cc)
                                    sps = psw.tile([128, 512], F32,
                                                   tag=f"st{l}",
                                                   name=f"st{l}")
                                    nc.tensor.matmul(
                                        sps[:, 0:ncols],
                                        qaug[p][64 * l:64 * l + R,
                                                128 * i:128 * i + 128],
                                        kaug[p][64 * l:64 * l + R,
                                                512 * cc:512 * cc + ncols],
                                        start=True, stop=True,
                                        tile_position=(64 * l, 0))
                                    if cc == nchunks - 1:
                                        a = ncols - 128
                                        nc.vector.tensor_add(
                                            sps[:, a:a + 128],
                                            sps[:, a:a + 128], triq_t[:])
                                    nc.vector.tensor_reduce(
                                        mx[:, 4 * h + cc:4 * h + cc + 1],
                                        sps[:, 0:ncols], axis=AXX, op=MAX)
                        for h in range(HPC):
                            nc.vector.tensor_reduce(
                                negm[:, h:h + 1],
                                mx[:, 4 * h:4 * h + nchunks],
                                axis=AXX, op=MAX, negate=True)
                        trout_bf = pmx.tile([128, 32], BF16, tag="troutb",
                                            name="trout_bf")
                        nc.vector.transpose(trout_bf[:], negm[:])
                        trout = pmx.tile([128, 32], F32R, tag="trout",
                                         name="trout")
                        nc.scalar.copy(trout[:], trout_bf[:])
                        for p in range(2):
                            for l in range(2):
                                h = 2 * p + l
                                for bb in range(4):
                                    nc.sync.dma_start(
                                        out=qaug[p][
                                            64 * l + R:64 * l + R + 1,
                                            128 * i + 32 * bb:
                                            128 * i + 32 * bb + 32],
                                        in_=trout[32 * bb + h:
                                                  32 * bb + h + 1, 0:32])

                def emit_stav(c):
                    njt = 4 * c + 4
                    avp = {}
                    for p in range(2):
                        for l in range(2):
                            avp[(p, l)] = psav.tile(
                                [DH + 1, 512], F32, tag=f"av{p}{l}",
                                name=f"av{p}{l}")
                    for j in range(njt):
                        for p in range(2):
                            for l in range(2):
                                h = 2 * p + l
                                stp = psT.tile([128, 512], F32,
                                               tag=f"sT{l}", name=f"sT{l}",
                                               bufs=1)
                                nc.tensor.matmul(
                                    stp[:],
                                    kaug[p][64 * l:64 * l + R + 17,
                                            128 * j:128 * j + 128],
                                    qaug[p][64 * l:64 * l + R + 17,
                                            512 * c:512 * c + 512],
                                    start=True, stop=True,
                                    tile_position=(64 * l, 0))
                                if j // 4 == c:
                                    a = 128 * (j - 4 * c)
                                    nc.vector.tensor_add(
                                        stp[:, a:a + 128],
                                        stp[:, a:a + 128], trik_t[:])
                                pt = pst.tile([128, 512], BF16, tag="pt",
                                              name="pt")
                                nc.scalar.activation(pt[:], stp[:], EXP)
                                nc.tensor.matmul(
                                    avp[(p, l)][:], vaug[h][j][:], pt[:],
                                    start=(j == 0), stop=(j == njt - 1))
                    for p in range(2):
                        lrow = pcx.tile([2, 512], F32R, tag="lrow",
                                        name="lrow")
                        ctxf = pcx.tile([128, 512], F32, tag="ctxf",
                                        name="ctxf")
                        for l in range(2):
                            l1 = pcx.tile([1, 512], F32R, tag=f"l1{l}",
                                          name=f"l1{l}")
                            nc.scalar.copy(l1[:], avp[(p, l)][DH:DH + 1, :])
                            nc.sync.dma_start(out=lrow[l:l + 1, :],
                                              in_=l1[:])
                            nc.scalar.copy(ctxf[64 * l:64 * l + 64, :],
                                           avp[(p, l)][0:DH, :])
                        scl = psw.tile([128, 512], F32, tag="st0",
                                       name="scl")
                        nc.tensor.matmul(scl[:], sel2_t[:], lrow[:],
                                         start=True, stop=True)
                        rinv = pcx.tile([128, 512], F32, tag="rinv",
                                        name="rinv")
                        nc.vector.reciprocal(rinv[:], scl[:])
                        nc.vector.tensor_mul(ctxr[p][c][:], ctxf[:],
                                             rinv[:])

                def emit_oproj(c):
                    for ot in range(8):
                        yps = psw.tile([128, 512], F32, tag="st1",
                                       name="yps")
                        for p in range(2):
                            nc.tensor.matmul(
                                yps[:],
                                wo_t[p][:, 128 * ot:128 * ot + 128],
                                ctxr[p][c][:],
                                start=(p == 0), stop=(p == 1))
                        ysb = pcx.tile([128, 512], F32, tag="ysb",
                                       name="ysb")
                        nc.scalar.copy(ysb[:], yps[:])
                        nc.sync.dma_start(
                            out=yT[128 * ot:128 * ot + 128,
                                   512 * c:512 * c + 512],
                            in_=ysb[:])

                emit_stats(0)
                emit_stats(1)
                for c in range(NCH):
                    if c + 2 < NCH:
                        emit_stats(c + 2)
                    emit_stav(c)
                    emit_oproj(c)

    nc.compile()
    return nc


def _consts():
    indq = np.zeros((NT, T), np.float32)
    for j in range(NT):
        indq[j, :128 * j] = NEG
    okq = np.zeros((NT + 1, T), np.float32)
    okq[0] = 1.0
    for j in range(NT):
        okq[1 + j, 128 * j:128 * j + 128] = 1.0
    triq = np.triu(np.full((128, 128), NEG, np.float32), 1)
    trik = np.tril(np.full((128, 128), NEG, np.float32), -1)
    sel2 = np.zeros((2, 128), np.float32)
    sel2[0, :64] = 1.0
    sel2[1, 64:] = 1.0
    ibf = np.eye(128).astype(ml_dtypes.bfloat16)
    return indq, okq, triq, trik, sel2, ibf


def kernel(x, Wq, bq, Wk, bk, Wv, bv, Wo, bo, Wq_lsr, Wk_lsr):
    from concourse.bass_utils import run_bass_kernel_spmd

    if "nc" not in _cache:
        _cache["nc"] = _build()
    nc = _cache["nc"]

    x = np.asarray(x, np.float32)
    Wq = np.asarray(Wq, np.float32)
    Wk = np.asarray(Wk, np.float32)
    Wv = np.asarray(Wv, np.float32)
    Wo = np.asarray(Wo, np.float32)
    bq = np.asarray(bq, np.float32)
    bk = np.asarray(bk, np.float32)
    bv = np.asarray(bv, np.float32)
    bo = np.asarray(bo, np.float32)
    Wq_lsr = np.asarray(Wq_lsr, np.float32)
    Wk_lsr = np.asarray(Wk_lsr, np.float32)

    indq, okq, triq, trik, sel2, ibf = _consts()
    in_maps = []
    for core in range(NCORES):
        b, g = divmod(core, 4)
        hs = HPC * g
        cols = slice(DH * hs, DH * hs + OC)
        # per-head lsr weights side by side: [DH, HPC*R]
        wql = np.ascontiguousarray(
            Wq_lsr[hs:hs + HPC].transpose(1, 0, 2).reshape(DH, HPC * R))
        wkl = np.ascontiguousarray(
            Wk_lsr[hs:hs + HPC].transpose(1, 0, 2).reshape(DH, HPC * R))
        in_maps.append({
            "xT": np.ascontiguousarray(x[b].T),
            "wq": np.ascontiguousarray(Wq[:, cols]),
            "wk": np.ascontiguousarray(Wk[:, cols]),
            "wv": np.ascontiguousarray(Wv[:, cols]),
            "wo": np.ascontiguousarray(Wo[cols, :]).astype(ml_dtypes.bfloat16),
            "wql": wql, "wkl": wkl,
            "bq": np.ascontiguousarray(bq[cols, None]),
            "bk": np.ascontiguousarray(bk[cols, None]),
            "indq": indq, "okq": okq, "triq": triq, "trik": trik,
            "sel2": sel2, "ibf": ibf,
        })

    res = run_bass_kernel_spmd(nc, in_maps, list(range(NCORES)),
                               **_cache.get("run_kwargs", {}))
    _cache["last_results"] = res

    y = np.zeros((B, T, D), np.float32)
    for core in range(NCORES):
        b = core // 4
        y[b] += res.results[core]["yT"].T
    y += (bv @ Wo + bo)[None, None, :]
    return y



# revision 13
# speedup vs baseline: 1.1005x; 1.1005x over previous
"""Multi-head low-rank-score (LSR) causal attention on 8 trn2 NeuronCores.

Sharding: core = 4*b + g owns batch b and heads [4g, 4g+4).

Key structure (v2 — HAM-warm redesign):
- Q/K projections are never materialized: q_lr^T = (Wq[:,cols] @ Wq_lsr)^T
  @ x^T, folded on the host into one [D,128] effective weight per side.
- Softmax max-stats pass: S in q-layout -> DVE row-max (fused diagonal
  mask via tensor_tensor_reduce) -> GpSimd max-tree -> PE transpose ->
  ScalarE negate-evac -> GpSimd scatter into the augmented -m rows.
  No DMA engine involvement.
- S^T pass: per (l-group, j): two quadrant matmuls into a 2-bank PSUM
  tile, one merged exp() on ScalarE (causally clipped via 3D APs),
  AV accumulates in 4 PSUM banks with the denominators coming from a
  leading ones-column in V.
- PSUM: 2x [128,2,512] S^T group tiles + 4x [65,512] AV accumulators
  = 8 banks; stats / o_proj / V-proj matmuls reuse the S^T slots in
  bursts at chunk boundaries so the PE queue never idles long enough
  for HAM to re-throttle the clock.
- Engine balance: ScalarE = exp + half the evacs; DVE = stats reduces,
  V/ctx evacs, fast-approx reciprocal; GpSimd = SBUF-side small ops.
"""

import numpy as np
import ml_dtypes

B = 2
T = 2048
D = 1024
H = 16
DH = 64
R = 32
HPC = 4  # heads per core
OC = HPC * DH  # 256 V-cols per core
NCORES = 8
SCALE = 1.0 / float(np.sqrt(np.float32(R)))
NEG = -30000.0
NT = T // 128  # 16 key/query tiles
NCH = T // 512  # 4 query chunks

_cache = {}


def _build():
    import concourse.bacc as bacc
    import concourse.mybir as mybir
    from concourse.tile import TileContext

    F32 = mybir.dt.float32
    F32R = mybir.dt.float32r
    BF16 = mybir.dt.bfloat16
    EXP = mybir.ActivationFunctionType.Exp
    COPY = mybir.ActivationFunctionType.Copy
    IDENT = mybir.ActivationFunctionType.Identity
    MAX = mybir.AluOpType.max
    ADD = mybir.AluOpType.add
    AXX = mybir.AxisListType.X

    nc = bacc.Bacc("TRN2", target_bir_lowering=False, debug=False,
                   num_devices=NCORES)

    xT = nc.declare_dram_parameter("xT", [D, T], F32R, isOutput=False)
    wv = nc.declare_dram_parameter("wv", [D, OC], F32R, isOutput=False)
    wqle = nc.declare_dram_parameter("wqle", [D, 128], F32R, isOutput=False)
    wkle = nc.declare_dram_parameter("wkle", [D, 128], F32R, isOutput=False)
    blq = nc.declare_dram_parameter("blq", [128, 1], F32, isOutput=False)
    blk = nc.declare_dram_parameter("blk", [128, 1], F32, isOutput=False)
    wo = nc.declare_dram_parameter("wo", [OC, D], BF16, isOutput=False)
    # [16, T] row j': NEG where t < 128*j' else 0 (q-side causal aug rows)
    indq = nc.declare_dram_parameter("indq", [NT, T], F32R, isOutput=False)
    # [17, T]: row 0 = ones; rows 1+j': 1.0 on k-tile j' cols else 0
    okq = nc.declare_dram_parameter("okq", [NT + 1, T], F32R, isOutput=False)
    # [128, 1024]: zeros, last 128 cols = triu(NEG, 1) (stats diag mask)
    tpad = nc.declare_dram_parameter("tpad", [128, 1024], F32, isOutput=False)
    # [128, 2, 128]: tril(NEG, -1) twice (S^T diag mask per group)
    trid2 = nc.declare_dram_parameter("trid2", [128, 2, 128], F32,
                                      isOutput=False)
    sel2 = nc.declare_dram_parameter("sel2", [2, 128], F32R, isOutput=False)
    idf = nc.declare_dram_parameter("idf", [128, 128], F32, isOutput=False)
    yT = nc.declare_dram_parameter("yT", [D, T], F32, isOutput=True)

    with TileContext(nc) as tc:
        with (
            nc.allow_low_precision(reason="f32r scores / bf16 P,V / approx recip"),
            tc.tile_pool(name="persist", bufs=1) as pp,
            tc.tile_pool(name="ps", bufs=1, space="PSUM") as ps,
            tc.tile_pool(name="work", bufs=2) as wk,
        ):
            # ---- persistent SBUF tiles
            wv_t = [pp.tile([128, OC], F32R, tag=f"wv{i}", name=f"wv{i}")
                    for i in range(8)]
            wqle_t = [pp.tile([128, 128], F32R, tag=f"wqle{i}", name=f"wqle{i}")
                      for i in range(8)]
            wkle_t = [pp.tile([128, 128], F32R, tag=f"wkle{i}", name=f"wkle{i}")
                      for i in range(8)]
            blq_t = pp.tile([128, 1], F32, tag="blq")
            blk_t = pp.tile([128, 1], F32, tag="blk")
            wo_t = [pp.tile([128, D], BF16, tag=f"wo{p}", name=f"wo{p}")
                    for p in range(2)]
            tpad_t = pp.tile([128, 1024], F32, tag="tpad")
            trid2_t = pp.tile([128, 2, 128], F32, tag="trid2")
            sel2_t = pp.tile([2, 128], F32R, tag="sel2")
            idf_t = pp.tile([128, 128], F32, tag="idf")
            # augmented lr tiles, one per head pair p (heads 2p, 2p+1)
            # rows [64l, 64l+32): q_lr^T/k_lr^T of head 2p+l (q side scaled)
            # row 64l+32: -m (q side) / ones (k side)
            # rows [64l+33, 64l+49): indq (q side) / k-tile selectors (k side)
            qaug = [pp.tile([128, T], F32R, tag=f"qaug{p}", name=f"qaug{p}")
                    for p in range(2)]
            kaug = [pp.tile([128, T], F32R, tag=f"kaug{p}", name=f"kaug{p}")
                    for p in range(2)]
            # V with leading ones column per head: [128, h, 0] = 1,
            # [128, h, 1:65] = V_h rows for this k-tile
            vaug = [pp.tile([128, HPC, DH + 1], BF16, tag=f"va{j}",
                            name=f"va{j}") for j in range(NT)]
            ctxr = [[pp.tile([128, 512], BF16, tag=f"cx{p}_{c}",
                             name=f"cx{p}_{c}") for c in range(NCH)]
                    for p in range(2)]
            xt_t = [pp.tile([128, T], F32R, tag=f"x{i}", name=f"x{i}")
                    for i in range(8)]

            # ---- input DMAs (ordered: lr weights -> x -> consts -> V/o)
            for i in range(8):
                nc.sync.dma_start(out=wqle_t[i][:],
                                  in_=wqle[128 * i:128 * i + 128, :])
                nc.sync.dma_start(out=wkle_t[i][:],
                                  in_=wkle[128 * i:128 * i + 128, :])
            nc.sync.dma_start(out=blq_t[:], in_=blq[:])
            nc.sync.dma_start(out=blk_t[:], in_=blk[:])
            for i in range(8):
                nc.sync.dma_start(out=xt_t[i][:],
                                  in_=xT[128 * i:128 * i + 128, :])
            nc.sync.dma_start(out=tpad_t[:], in_=tpad[:])
            nc.sync.dma_start(out=trid2_t[:], in_=trid2[:])
            nc.sync.dma_start(out=sel2_t[:], in_=sel2[:])
            nc.sync.dma_start(out=idf_t[:], in_=idf[:])
            for p in range(2):
                for l in range(2):
                    nc.sync.dma_start(
                        out=qaug[p][64 * l + 33:64 * l + 49, :], in_=indq[:])
                    nc.sync.dma_start(
                        out=kaug[p][64 * l + 32:64 * l + 49, :], in_=okq[:])
            for i in range(8):
                nc.sync.dma_start(out=wv_t[i][:],
                                  in_=wv[128 * i:128 * i + 128, :])
            for p in range(2):
                nc.sync.dma_start(out=wo_t[p][:],
                                  in_=wo[128 * p:128 * p + 128, :])
            # ones columns of vaug (constant, written once)
            for j in range(NT):
                nc.gpsimd.memset(vaug[j][:, :, DH:DH + 1], 1.0)

            # ---- PSUM slot machinery: 2 group tags (2 banks each) + 4 AV
            slot_ctr = [0]

            def gslot():
                """Next [128, 512] PSUM scratch view, round-robin over the
                4 half-group slots."""
                i = slot_ctr[0] % 4
                slot_ctr[0] += 1
                g = ps.tile([128, 2, 512], F32, tag=f"g{i // 2}",
                            name=f"g{i // 2}")
                return g[:, i % 2, :]

            # ---- phase P: lr projections + V projection
            aug = (qaug, kaug)
            wle = (wqle_t, wkle_t)
            bl = (blq_t, blk_t)
            for ch in range(NCH):
                for side in range(2):
                    v = gslot()
                    for kk in range(8):
                        nc.tensor.matmul(
                            v, wle[side][kk][:],
                            xt_t[kk][:, 512 * ch:512 * ch + 512],
                            start=(kk == 0), stop=(kk == 7))
                    # 4 partition-block evacs -> aug lr rows (+ lr bias)
                    for u in range(4):
                        p, l = u % 2, u // 2
                        dst = aug[side][p][64 * l:64 * l + 32,
                                           512 * ch:512 * ch + 512]
                        src = v[32 * u:32 * u + 32, :]
                        bias = bl[side][32 * u:32 * u + 32, :]
                        if u < 2:
                            nc.scalar.activation(dst, src, IDENT, bias=bias,
                                                 scale=1.0)
                        else:
                            nc.vector.tensor_scalar_add(dst, src, bias)
            for tt in range(NT):
                v = gslot()
                for kk in range(8):
                    nc.tensor.matmul(
                        v[:, 0:OC], xt_t[kk][:, 128 * tt:128 * tt + 128],
                        wv_t[kk][:], start=(kk == 0), stop=(kk == 7))
                nc.vector.tensor_copy(
                    vaug[tt][:, :, 0:DH],
                    v[:, 0:OC].rearrange("p (h d) -> p h d", h=HPC))

            # ---- stats burst: row maxes -> -m rows of qaug
            def emit_stats(c):
                for i in range(4 * c, 4 * c + 4):
                    nch = i // 4 + 1
                    mxt = wk.tile([128, 16], F32, tag="mx", name="mx")
                    for cc in range(nch):
                        ncols = min(512, 128 * (i + 1) - 512 * cc)
                        for u in range(4):
                            p, l = u % 2, u // 2
                            v = gslot()
                            nc.tensor.matmul(
                                v[:, 0:ncols],
                                qaug[p][64 * l:64 * l + R,
                                        128 * i:128 * i + 128],
                                kaug[p][64 * l:64 * l + R,
                                        512 * cc:512 * cc + ncols],
                                start=True, stop=True,
                                tile_position=(64 * l, 0))
                            mcol = mxt[:, 4 * cc + u:4 * cc + u + 1]
                            if cc == nch - 1:
                                nc.vector.tensor_add(
                                    v[:, ncols - 128:ncols],
                                    v[:, ncols - 128:ncols],
                                    tpad_t[:, 896:1024])
                            nc.vector.tensor_reduce(
                                mcol, v[:, 0:ncols], axis=AXX, op=MAX)
                    t4 = wk.tile([128, 4], F32, tag="t4", name="t4")
                    nc.vector.tensor_reduce(
                        t4[:],
                        mxt[:, 0:4 * nch].rearrange("p (c u) -> p u c", u=4),
                        axis=AXX, op=MAX)
                    msrc = t4[:]
                    pv = gslot()
                    nc.tensor.transpose(pv[0:4, 0:128], msrc, idf_t[:])
                    ns = wk.tile([4, 128], F32R, tag="ns", name="ns")
                    nc.scalar.activation(ns[:], pv[0:4, 0:128], COPY,
                                         scale=-1.0)
                    for u in range(4):
                        p, l = u % 2, u // 2
                        nc.sync.dma_start(
                            out=qaug[p][64 * l + 32:64 * l + 33,
                                        128 * i:128 * i + 128],
                            in_=ns[u:u + 1, :])

            # ---- T(c): S^T + exp + AV for one 512-query chunk
            def emit_T(c):
                njt = 4 * c + 4
                av = {}
                for p in range(2):
                    for l in range(2):
                        av[(p, l)] = ps.tile([DH + 1, 512], F32,
                                             tag=f"a{p}{l}", name=f"a{p}{l}")
                for j in range(njt):
                    dd = j - 4 * c
                    # causal clip: columns [0, am) of this j-tile are fully
                    # masked; skip them (am=256 for dd=3: f32r needs F>=256)
                    am = (0, 128, 256, 256)[dd] if dd >= 0 else 0
                    pts = []
                    for l in range(2):
                        g = ps.tile([128, 2, 512], F32, tag=f"g{l}",
                                    name=f"g{l}")
                        for p in range(2):
                            nc.tensor.matmul(
                                g[:, p, am:512],
                                kaug[p][64 * l:64 * l + R + 17,
                                        128 * j:128 * j + 128],
                                qaug[p][64 * l:64 * l + R + 17,
                                        512 * c + am:512 * c + 512],
                                start=True, stop=True,
                                tile_position=(64 * l, 0))
                        if dd >= 0:
                            d0 = 128 * dd
                            for p in range(2):
                                nc.vector.tensor_add(g[:, p, d0:d0 + 128],
                                                     g[:, p, d0:d0 + 128],
                                                     trid2_t[:, 0, :])
                        pt = wk.tile([128, 2, 512], BF16, tag=f"pt{l}",
                                     name=f"pt{l}")
                        for p in range(2):
                            nc.scalar.activation(pt[:, p, am:512],
                                                 g[:, p, am:512], EXP)
                        pts.append(pt)
                    for l in range(2):
                        for p in range(2):
                            h = 2 * p + l
                            nc.tensor.matmul(
                                av[(p, l)][:, am:512], vaug[j][:, h, :],
                                pts[l][:, p, am:512],
                                start=(j == 0), stop=(j == njt - 1))
                # epilogue: denominator rows -> broadcast -> 1/x -> ctx
                for p in range(2):
                    rd = wk.tile([2, 512], F32, tag=f"rd{p}", name=f"rd{p}")
                    for l in range(2):
                        dl = wk.tile([1, 512], F32, tag=f"dl{l}",
                                     name=f"dl{l}")
                        nc.scalar.copy(dl[:], av[(p, l)][DH:DH + 1, :])
                        nc.sync.dma_start(out=rd[l:l + 1, :], in_=dl[:])
                    bv_ = gslot()
                    nc.tensor.matmul(bv_, sel2_t[:], rd[:].bitcast(F32R),
                                     start=True, stop=True)
                    rv = wk.tile([128, 512], F32, tag="rv", name="rv")
                    nc.vector.reciprocal_approx_fast(out=rv[:], in_=bv_)
                    cf = wk.tile([128, 512], F32, tag="cf", name="cf")
                    for l in range(2):
                        nc.vector.tensor_copy(cf[64 * l:64 * l + 64, :],
                                              av[(p, l)][0:DH, :])
                    nc.vector.tensor_mul(ctxr[p][c][:], cf[:], rv[:])

            def emit_oproj(c):
                for ot in range(8):
                    y = gslot()
                    for p in range(2):
                        nc.tensor.matmul(
                            y, wo_t[p][:, 128 * ot:128 * ot + 128],
                            ctxr[p][c][:], start=(p == 0), stop=(p == 1))
                    ysb = wk.tile([128, 512], F32, tag="ysb", name="ysb")
                    if ot % 2 == 0:
                        nc.scalar.copy(ysb[:], y)
                    else:
                        nc.vector.tensor_copy(ysb[:], y)
                    nc.sync.dma_start(
                        out=yT[128 * ot:128 * ot + 128,
                               512 * c:512 * c + 512],
                        in_=ysb[:])

            emit_stats(0)
            emit_stats(1)
            for c in range(NCH):
                emit_T(c)
                if c + 2 < NCH:
                    emit_stats(c + 2)
                emit_oproj(c)

    nc.compile()
    return nc


def _consts():
    indq = np.zeros((NT, T), np.float32)
    for j in range(NT):
        indq[j, :128 * j] = NEG
    okq = np.zeros((NT + 1, T), np.float32)
    okq[0] = 1.0
    for j in range(NT):
        okq[1 + j, 128 * j:128 * j + 128] = 1.0
    tpad = np.zeros((128, 1024), np.float32)
    tpad[:, 896:1024] = np.triu(np.full((128, 128), NEG, np.float32), 1)
    trik = np.tril(np.full((128, 128), NEG, np.float32), -1)
    trid2 = np.stack([trik, trik], axis=1)  # [128, 2, 128]
    sel2 = np.zeros((2, 128), np.float32)
    sel2[0, :64] = 1.0
    sel2[1, 64:] = 1.0
    idf = np.eye(128, dtype=np.float32)
    return indq, okq, tpad, trid2, sel2, idf


def kernel(x, Wq, bq, Wk, bk, Wv, bv, Wo, bo, Wq_lsr, Wk_lsr):
    from concourse.bass_utils import run_bass_kernel_spmd

    if "nc" not in _cache:
        _cache["nc"] = _build()
    nc = _cache["nc"]

    x = np.asarray(x, np.float32)
    Wq = np.asarray(Wq, np.float32)
    Wk = np.asarray(Wk, np.float32)
    Wv = np.asarray(Wv, np.float32)
    Wo = np.asarray(Wo, np.float32)
    bq = np.asarray(bq, np.float32)
    bk = np.asarray(bk, np.float32)
    bv = np.asarray(bv, np.float32)
    bo = np.asarray(bo, np.float32)
    Wq_lsr = np.asarray(Wq_lsr, np.float32)
    Wk_lsr = np.asarray(Wk_lsr, np.float32)

    indq, okq, tpad, trid2, sel2, idf = _consts()
    # local head order for the lr-psum partition blocks: u=(p,l) -> h=2p+l
    horder = [0, 2, 1, 3]
    in_maps = []
    for core in range(NCORES):
        b, g = divmod(core, 4)
        hs = HPC * g
        cols = slice(DH * hs, DH * hs + OC)
        # effective low-rank projection weights: [D, 4*R], block u = head
        # horder[u]; q side carries the 1/sqrt(R) score scale
        wqle_blocks, wkle_blocks, blq_v, blk_v = [], [], [], []
        for u in range(4):
            h = hs + horder[u]
            wq_h = Wq[:, DH * h:DH * h + DH] @ Wq_lsr[h] * SCALE
            wk_h = Wk[:, DH * h:DH * h + DH] @ Wk_lsr[h]
            wqle_blocks.append(wq_h)
            wkle_blocks.append(wk_h)
            blq_v.append(bq[DH * h:DH * h + DH] @ Wq_lsr[h] * SCALE)
            blk_v.append(bk[DH * h:DH * h + DH] @ Wk_lsr[h])
        in_maps.append({
            "xT": np.ascontiguousarray(x[b].T),
            "wv": np.ascontiguousarray(Wv[:, cols]),
            "wqle": np.ascontiguousarray(np.concatenate(wqle_blocks, axis=1)),
            "wkle": np.ascontiguousarray(np.concatenate(wkle_blocks, axis=1)),
            "blq": np.concatenate(blq_v).reshape(128, 1).astype(np.float32),
            "blk": np.concatenate(blk_v).reshape(128, 1).astype(np.float32),
            "wo": np.ascontiguousarray(Wo[cols, :]).astype(ml_dtypes.bfloat16),
            "indq": indq, "okq": okq, "tpad": tpad, "trid2": trid2,
            "sel2": sel2, "idf": idf,
        })

    res = run_bass_kernel_spmd(nc, in_maps, list(range(NCORES)),
                               **_cache.get("run_kwargs", {}))
    _cache["last_results"] = res

    y = np.zeros((B, T, D), np.float32)
    for core in range(NCORES):
        b = core // 4
        y[b] += res.results[core]["yT"].T
    y += (bv @ Wo + bo)[None, None, :]
    return y
